# revision 5
# baseline (speedup 1.0000x reference)
"""Trainium2 Bass kernel for the CriticBaseline problem.

reference:
    G = discounted_returns(rewards)            # reverse scan, gamma=0.99
    h = relu(obs @ W1 + b1); h = relu(h @ W2 + b2)
    V = (h @ W3 + b3).reshape(-1)
    return G - V                               # [T]

Strategy (8 NeuronCores, SPMD, no collectives):
  - Data-parallel over T: core c owns timesteps [c*8192, (c+1)*8192).
  - MLP GEMMs run on the PE array in float32r (1 cyc/row at N=512) with
    weights resident in SBUF; obs is fed pre-transposed ([D, Tc]) so the
    contraction dim lands on partitions; activations stay on-chip in
    [H-part, T-free] layout through all three layers.
  - The discounted-return scan is computed per-core as a *banded* matmul:
    gamma^k decays below 1.2e-9 by k=2048, so G[i] only needs the next
    2048 rewards. Each core gets its reward slice plus a 2048 overlap
    (zero-padded at the global end) and computes
        G[128b+p] = sum_j (Mj^T @ rmat[:, b+j])[p]
    with 17 host-precomputed [128,128] coefficient matrices Mj in fp32.
    This replaces the serial scan and the inter-core carry entirely;
    truncation error is ~1e-9 relative.
"""

import numpy as np

GAMMA = 0.99
T, D, H = 65536, 1024, 1024
N_CORES = 8
TC = T // N_CORES  # 8192 timesteps per core
TT = 512           # moving-dim tile (max for 4-byte matmul)
NT = TC // TT      # 16 t-tiles per core
NB = TC // 128     # 64 blocks of 128 timesteps
WIN = 2048         # scan window: gamma^2048 ~ 1.1e-9
NJ = WIN // 128    # 16 -> coefficient matrices j = 0..16
RCOLS = NB + NJ    # 80 columns of packed rewards per core

_cache = {}


def _scan_mats() -> np.ndarray:
    """Mj[k, p] = gamma^(128j + k - p) on the band 0 <= 128j+k-p < WIN."""
    k = np.arange(128)[:, None]
    p = np.arange(128)[None, :]
    mats = []
    for j in range(NJ + 1):
        e = 128 * j + k - p
        m = np.where((e >= 0) & (e < WIN), np.power(GAMMA, e, dtype=np.float64), 0.0)
        mats.append(m.astype(np.float32))
    return np.ascontiguousarray(np.stack(mats))


def _build():
    """Build + schedule the single-core SPMD Bass program (cached)."""
    if "nc" in _cache:
        return _cache["nc"]

    from contextlib import ExitStack

    import concourse.mybir as mybir
    import concourse.tile as tile
    from concourse import bacc
    from concourse.tile_rust import add_dep_helper

    f32 = mybir.dt.float32
    f32r = mybir.dt.float32r
    Relu = mybir.ActivationFunctionType.Relu
    Copy = mybir.ActivationFunctionType.Copy

    nc = bacc.Bacc("TRN2", target_bir_lowering=False, debug=False, num_devices=N_CORES)

    obsT = nc.dram_tensor("obsT", [D, TC], f32r, kind="ExternalInput").ap()
    w1 = nc.dram_tensor("w1", [D, H], f32r, kind="ExternalInput").ap()
    w2 = nc.dram_tensor("w2", [H, H], f32r, kind="ExternalInput").ap()
    w3p = nc.dram_tensor("w3p", [128, 8], f32r, kind="ExternalInput").ap()
    b1p = nc.dram_tensor("b1p", [128, 8], f32, kind="ExternalInput").ap()
    b2p = nc.dram_tensor("b2p", [128, 8], f32, kind="ExternalInput").ap()
    b3r = nc.dram_tensor("b3r", [1, 1], f32, kind="ExternalInput").ap()
    rmat = nc.dram_tensor("rmat", [128, RCOLS], f32, kind="ExternalInput").ap()
    scanm = nc.dram_tensor("scanm", [NJ + 1, 128, 128], f32, kind="ExternalInput").ap()
    out = nc.dram_tensor("out", [TC], f32, kind="ExternalOutput").ap()

    with tile.TileContext(nc) as tc, ExitStack() as ctx:
        const = ctx.enter_context(tc.tile_pool(name="const", bufs=1))
        w1_sb = const.tile([128, 8, H], f32r, name="w1_sb")   # [p, dj, h]
        w2_sb = const.tile([128, 8, H], f32r, name="w2_sb")   # [p, hj, h]
        scan_sb = const.tile([128, NJ + 1, 128], f32, name="scan_sb")
        rmat_sb = const.tile([128, RCOLS], f32, name="rmat_sb")
        w3_sb = const.tile([128, 8], f32r, name="w3_sb")
        b1_sb = const.tile([128, 8], f32, name="b1_sb")
        b2_sb = const.tile([128, 8], f32, name="b2_sb")
        b3_sb = const.tile([1, 1], f32, name="b3_sb")

        nc.sync.dma_start(scan_sb[:], scanm.rearrange("j p m -> p j m"))
        nc.sync.dma_start(rmat_sb[:], rmat[:])
        nc.sync.dma_start(w3_sb[:], w3p[:])
        nc.sync.dma_start(b1_sb[:], b1p[:])
        nc.sync.dma_start(b2_sb[:], b2p[:])
        nc.sync.dma_start(b3_sb[:], b3r[:])
        nc.sync.dma_start(w1_sb[:], w1.rearrange("(j p) h -> p j h", p=128))
        nc.sync.dma_start(w2_sb[:], w2.rearrange("(j p) h -> p j h", p=128))

        # ---- discounted returns: banded matmul, fp32 ----
        gps = ctx.enter_context(tc.tile_pool(name="gps", bufs=1, space="PSUM"))
        g_psum = gps.tile([128, NB], f32, name="g_psum")
        for j in range(NJ + 1):
            nc.tensor.matmul(
                g_psum[:],
                lhsT=scan_sb[:, j, :],
                rhs=rmat_sb[:, j : j + NB],
                start=(j == 0),
                stop=(j == NJ),
            )
        gsbp = ctx.enter_context(tc.tile_pool(name="gsbp", bufs=1))
        g_sb = gsbp.tile([128, NB], f32, name="g_sb")
        nc.scalar.activation(g_sb[:], g_psum[:], Copy)

        # ---- MLP over t-tiles, float32r matmuls ----
        otp = ctx.enter_context(tc.tile_pool(name="otp", bufs=16))
        h1p = ctx.enter_context(tc.tile_pool(name="h1p", bufs=16))
        h2p = ctx.enter_context(tc.tile_pool(name="h2p", bufs=16))
        ps1 = ctx.enter_context(tc.tile_pool(name="ps1", bufs=2, space="PSUM"))
        ps2 = ctx.enter_context(tc.tile_pool(name="ps2", bufs=2, space="PSUM"))
        ps3 = ctx.enter_context(tc.tile_pool(name="ps3", bufs=2, space="PSUM"))
        vsp = ctx.enter_context(tc.tile_pool(name="vsp", bufs=2))
        vdp = ctx.enter_context(tc.tile_pool(name="vdp", bufs=1, space="DRAM"))
        vdram = vdp.tile([NT, TT], f32, name="vdram")

        v_dmas = []
        for it in range(NT):
            t0 = it * TT
            ots = []
            for dj in range(8):
                ot = otp.tile([128, TT], f32r, tag="ot", name=f"ot_{it}_{dj}")
                nc.sync.dma_start(ot[:], obsT[dj * 128 : (dj + 1) * 128, t0 : t0 + TT])
                ots.append(ot)

            h1s = []
            for ho in range(8):
                p1 = ps1.tile([128, TT], f32, tag="p1", name=f"p1_{it}_{ho}")
                for dj in range(8):
                    nc.tensor.matmul(
                        p1[:],
                        lhsT=w1_sb[:, dj, ho * 128 : (ho + 1) * 128],
                        rhs=ots[dj][:],
                        start=(dj == 0),
                        stop=(dj == 7),
                    )
                h1 = h1p.tile([128, TT], f32r, tag="h1", name=f"h1_{it}_{ho}")
                nc.scalar.activation(h1[:], p1[:], Relu, bias=b1_sb[:, ho : ho + 1])
                h1s.append(h1)

            h2s = []
            for ho in range(8):
                p2 = ps2.tile([128, TT], f32, tag="p2", name=f"p2_{it}_{ho}")
                for hj in range(8):
                    nc.tensor.matmul(
                        p2[:],
                        lhsT=w2_sb[:, hj, ho * 128 : (ho + 1) * 128],
                        rhs=h1s[hj][:],
                        start=(hj == 0),
                        stop=(hj == 7),
                    )
                h2 = h2p.tile([128, TT], f32r, tag="h2", name=f"h2_{it}_{ho}")
                nc.scalar.activation(h2[:], p2[:], Relu, bias=b2_sb[:, ho : ho + 1])
                h2s.append(h2)

            p3 = ps3.tile([1, TT], f32, tag="p3", name=f"p3_{it}")
            for hj in range(8):
                nc.tensor.matmul(
                    p3[:],
                    lhsT=w3_sb[:, hj : hj + 1],
                    rhs=h2s[hj][:],
                    start=(hj == 0),
                    stop=(hj == 7),
                )
            vseg = vsp.tile([1, TT], f32, tag="vseg", name=f"vseg_{it}")
            nc.scalar.activation(
                vseg[:], p3[:], mybir.ActivationFunctionType.Identity, bias=b3_sb[:]
            )
            v_dmas.append(nc.sync.dma_start(vdram[it : it + 1, :], vseg[0:1, :]))

        # ---- final: out[128b+p] = G[p,b] - V[p,b] ----
        finp = ctx.enter_context(tc.tile_pool(name="finp", bufs=1))
        vmat = finp.tile([128, NB], f32, name="vmat")
        gather = nc.sync.dma_start(
            vmat[:], vdram[:].rearrange("n (b2 p) -> p (n b2)", p=128)
        )
        for vd in v_dmas:
            add_dep_helper(gather.ins, vd.ins, reason="v gather after v segment writes")
        omat = finp.tile([128, NB], f32, name="omat")
        nc.vector.tensor_sub(omat[:], g_sb[:], vmat[:])
        nc.sync.dma_start(out.rearrange("(b p) -> p b", p=128), omat[:])

    nc.compile()
    _cache["nc"] = nc
    return nc


def kernel(rewards, obs, W1, b1, W2, b2, W3, b3):
    from concourse.bass_utils import run_bass_kernel_spmd

    rewards = np.asarray(rewards, dtype=np.float32)
    obs = np.asarray(obs, dtype=np.float32)
    W1 = np.ascontiguousarray(np.asarray(W1, dtype=np.float32))
    W2 = np.ascontiguousarray(np.asarray(W2, dtype=np.float32))
    W3 = np.asarray(W3, dtype=np.float32)
    b1 = np.asarray(b1, dtype=np.float32)
    b2 = np.asarray(b2, dtype=np.float32)
    b3 = np.asarray(b3, dtype=np.float32)

    nc = _build()

    scanm = _scan_mats()
    w3p = np.ascontiguousarray(W3.reshape(8, 128).T)
    b1p = np.ascontiguousarray(b1.reshape(8, 128).T)
    b2p = np.ascontiguousarray(b2.reshape(8, 128).T)
    b3r = np.ascontiguousarray(b3.reshape(1, 1))

    r_pad = np.zeros(T + WIN, dtype=np.float32)
    r_pad[:T] = rewards

    in_maps = []
    for c in range(N_CORES):
        lo = c * TC
        obsT_c = np.ascontiguousarray(obs[lo : lo + TC].T)
        rmat_c = np.ascontiguousarray(
            r_pad[lo : lo + TC + WIN].reshape(RCOLS, 128).T
        )
        in_maps.append(
            {
                "obsT": obsT_c,
                "w1": W1,
                "w2": W2,
                "w3p": w3p,
                "b1p": b1p,
                "b2p": b2p,
                "b3r": b3r,
                "rmat": rmat_c,
                "scanm": scanm,
            }
        )

    res = run_bass_kernel_spmd(nc, in_maps, core_ids=list(range(N_CORES)))
    return np.concatenate([res.results[c]["out"] for c in range(N_CORES)])
